# revision 1
# baseline (speedup 1.0000x reference)
"""Trainium2 kernel for nn_ClusteringLayer (vq_codebook).

Problem: x (1, 131072, 256) f32, cluster_centers (1024, 256) f32.
For each cluster k: find argmin_n ||x[n] - c[k]||^2 and return that x row.
Output: (1, 1024, 256) f32.

Strategy (8 NeuronCores, x sharded along n, centers replicated):
  argmin_n d2[n,k] == argmax_n s[n,k],  s = 2*x.c - |x|^2  (c2[k] const per k)
  Host pre-sorts points by |x|^2, so |x|^2 is nearly constant inside each
  contiguous 2048-point group. The device then needs no x2 at all:
    psum[k_tile, grp] = bf16 matmul  xT_sorted (moving) x (2C)T (stationary)
    VectorE reduce_max over each group directly from PSUM -> bmax2dot f32.
  Host recovery per cluster:
    upper/lower bounds of the true group max of s from bmax2dot and the
    group's [x2min, x2max]; every group whose upper bound reaches the best
    lower bound - THETA is rescored exactly (fp32 gemm + fp64 refine,
    first-original-index tiebreak). Exactness relies only on bounds +
    THETA covering the bf16 matmul noise (~0.12 abs, validated).
"""

import os
import sys

for _p in ("/opt/trn_rl_repo",):
    if os.path.isdir(_p) and _p not in sys.path:
        sys.path.append(_p)

import numpy as np
import ml_dtypes

import concourse.bass as bass
import concourse.bacc as bacc
import concourse.mybir as mybir
import concourse.tile as tile

NCORES = 8
N = 131072
F = 256
K = 1024
SH = N // NCORES            # 16384 points per core
GRP = 2048                  # group size for the device-side max reduction
NG = SH // GRP              # 8 groups per core
NGRP = NCORES * NG          # 64 groups total
KT = K // 128               # 8 cluster tiles
NCH = F // 128              # 2 contraction chunks
THETA = 2.5                 # host rescue radius (covers bf16 score noise)
TOPM = 32                   # fp32->fp64 refine width per (cluster, group)

BF16 = ml_dtypes.bfloat16


def build_nc():
    """Build + compile the per-core Bass program (same program on all cores)."""
    nc = bacc.Bacc("TRN2", target_bir_lowering=False, debug=False,
                   num_devices=NCORES)

    xt = nc.dram_tensor("xt", [NCH, 128, SH], mybir.dt.bfloat16,
                        kind="ExternalInput")
    ct2 = nc.dram_tensor("ct2", [NCH, 128, K], mybir.dt.bfloat16,
                         kind="ExternalInput")
    bmax_d = nc.dram_tensor("bmax", [128, KT * NG], mybir.dt.float32,
                            kind="ExternalOutput")

    with tile.TileContext(nc) as tc:
        with (
            tc.tile_pool(name="consts", bufs=1) as cpool,
            tc.tile_pool(name="xtp", bufs=3) as xpool,
            tc.tile_pool(name="psum", bufs=2, space="PSUM") as ppool,
            tc.tile_pool(name="scrap", bufs=3) as spool,
        ):
            warm_w = cpool.tile([128, 128], mybir.dt.bfloat16, tag="warmw")
            warm_x = cpool.tile([128, 512], mybir.dt.bfloat16, tag="warmx")
            nc.gpsimd.memset(warm_w[:], 0.0)
            nc.gpsimd.memset(warm_x[:], 0.0)
            warm_ps = ppool.tile([128, 512], mybir.dt.float32, tag="ps",
                                 name="warmps")
            for _ in range(24):
                nc.tensor.matmul(warm_ps[:], lhsT=warm_w[:], rhs=warm_x[:],
                                 start=True, stop=True)

            ct2_t = []
            for ch in range(NCH):
                t = cpool.tile([128, K], mybir.dt.bfloat16, tag=f"ct{ch}")
                for h in range(2):
                    nc.sync.dma_start(t[:, h * K // 2:(h + 1) * K // 2],
                                      ct2[ch, :, h * K // 2:(h + 1) * K // 2])
                ct2_t.append(t)
            bmax_t = cpool.tile([128, KT * NG], mybir.dt.float32, tag="bmax")

            for g in range(NG):
                # per-512-block x tiles: finer DMA granularity lets the first
                # matmuls start as soon as one 128KB slice lands
                xg = []
                for ch in range(NCH):
                    blks = []
                    for blk in range(GRP // 512):
                        t = xpool.tile([128, 512], mybir.dt.bfloat16,
                                       tag=f"xt{ch}b{blk}")
                        base = g * GRP + blk * 512
                        nc.sync.dma_start(t[:], xt[ch, :, base:base + 512])
                        blks.append(t)
                    xg.append(blks)

                for kt in range(KT):
                    ps = ppool.tile([128, GRP], mybir.dt.float32, tag="ps")
                    for ch in range(NCH):
                        for blk in range(GRP // 512):
                            nc.tensor.matmul(
                                ps[:, blk * 512:(blk + 1) * 512],
                                lhsT=ct2_t[ch][:, kt * 128:(kt + 1) * 128],
                                rhs=xg[ch][blk][:],
                                start=(ch == 0),
                                stop=(ch == NCH - 1),
                            )
                    if True:
                        col = kt * NG + g
                        # ~1 direct-psum reduce per group, rest evac+fold
                        if kt == (g % KT):
                            nc.vector.tensor_reduce(
                                out=bmax_t[:, col:col + 1],
                                in_=ps[:],
                                axis=mybir.AxisListType.X,
                                op=mybir.AluOpType.max,
                            )
                        else:
                            ev = spool.tile([128, GRP], mybir.dt.float16,
                                            tag="ev")
                            nc.scalar.copy(ev[:], ps[:])
                            f1 = spool.tile([128, GRP // 2], mybir.dt.float16,
                                            tag="f1")
                            nc.vector.tensor_tensor(
                                out=f1[:], in0=ev[:, 0:GRP // 2],
                                in1=ev[:, GRP // 2:GRP],
                                op=mybir.AluOpType.max)
                            f2 = spool.tile([128, GRP // 4], mybir.dt.float16,
                                            tag="f2")
                            nc.vector.tensor_tensor(
                                out=f2[:], in0=f1[:, 0:GRP // 4],
                                in1=f1[:, GRP // 4:GRP // 2],
                                op=mybir.AluOpType.max)
                            f3 = spool.tile([128, GRP // 8], mybir.dt.float16,
                                            tag="f3")
                            nc.vector.tensor_tensor(
                                out=f3[:], in0=f2[:, 0:GRP // 8],
                                in1=f2[:, GRP // 8:GRP // 4],
                                op=mybir.AluOpType.max)
                            nc.vector.tensor_reduce(
                                out=bmax_t[:, col:col + 1],
                                in_=f3[:],
                                axis=mybir.AxisListType.X,
                                op=mybir.AluOpType.max,
                            )

            nc.sync.dma_start(bmax_d[:, :], bmax_t[:])

    nc.compile()
    return nc


def host_prep(x, cluster_centers):
    """Sort points by |x|^2; build per-core device inputs."""
    x0 = np.ascontiguousarray(x[0], dtype=np.float32)        # (N, F)
    C = np.ascontiguousarray(cluster_centers, dtype=np.float32)
    x2 = np.einsum('nf,nf->n', x0.astype(np.float64),
                   x0.astype(np.float64))
    order = np.argsort(x2, kind="stable").astype(np.int64)
    xs_all = x0[order]                                        # sorted points
    x2s = x2[order]
    ct2_np = np.ascontiguousarray(
        (2.0 * C).T.astype(BF16)).reshape(NCH, 128, K)
    in_maps = []
    for c in range(NCORES):
        xs = xs_all[c * SH:(c + 1) * SH]
        xt_np = np.ascontiguousarray(xs.T.astype(BF16)).reshape(NCH, 128, SH)
        in_maps.append({"xt": xt_np, "ct2": ct2_np})
    return in_maps, x0, C, order, xs_all, x2s


def host_combine(bmax_cores, x0, C, order, xs_all, x2s):
    """Exact argmin recovery from per-group maxima of 2*dot (sorted points)."""
    x64s = xs_all.astype(np.float64)
    C64 = C.astype(np.float64)
    x2s_32 = x2s.astype(np.float32)

    # bmax_cores[c]: [128, KT*NG] -> cluster k = kt*128 + p, col = kt*NG + g
    bm = np.empty((K, NGRP), dtype=np.float32)
    for c in range(NCORES):
        a = np.asarray(bmax_cores[c]).reshape(128, KT, NG)
        bm[:, c * NG:(c + 1) * NG] = a.transpose(1, 0, 2).reshape(K, NG)

    gb = np.arange(NGRP) * GRP
    x2min = x2s[gb].astype(np.float32)            # sorted -> min is first
    x2max = x2s[gb + GRP - 1].astype(np.float32)

    ub = bm - x2min[None, :]                      # >= true group smax
    lb = bm - x2max[None, :]                      # <= true group smax
    win_lb = lb.max(axis=1)
    flags = ub >= (win_lb[:, None] - THETA)       # (K, NGRP)

    pair_clusters = [[] for _ in range(NGRP)]
    ks_idx, ps_idx = np.nonzero(flags)
    for kk, p in zip(ks_idx, ps_idx):
        pair_clusters[p].append(kk)

    best_val = np.full(K, np.inf)
    best_idx = np.zeros(K, dtype=np.int64)        # original indices
    for p, ks in enumerate(pair_clusters):
        if not ks:
            continue
        base = p * GRP
        pts = xs_all[base:base + GRP]
        d32 = x2s_32[base:base + GRP, None] - 2.0 * (pts @ C[ks].T)
        m = min(TOPM, GRP - 1)
        part = np.argpartition(d32, m, axis=0)[:m]
        for j, kk in enumerate(ks):
            srt = base + part[:, j]
            dv = x2s[srt] - 2.0 * (x64s[srt] @ C64[kk])
            ids = order[srt]                      # original indices
            o = np.lexsort((ids, dv))[0]
            if (dv[o] < best_val[kk]) or (dv[o] == best_val[kk]
                                          and ids[o] < best_idx[kk]):
                best_val[kk] = dv[o]
                best_idx[kk] = ids[o]

    return x0[best_idx][None].astype(np.float32)


_NC_CACHE = {}


def kernel(x, cluster_centers):
    from concourse.bass_utils import run_bass_kernel_spmd

    if "nc" not in _NC_CACHE:
        _NC_CACHE["nc"] = build_nc()
    nc = _NC_CACHE["nc"]

    in_maps, x0, C, order, xs_all, x2s = host_prep(x, cluster_centers)
    res = run_bass_kernel_spmd(nc, in_maps, list(range(NCORES)))
    bmax_cores = [res.results[c]["bmax"] for c in range(NCORES)]
    return host_combine(bmax_cores, x0, C, order, xs_all, x2s)



# revision 2
# speedup vs baseline: 1.0260x; 1.0260x over previous
"""Trainium2 kernel v2 for nn_ClusteringLayer (vq_codebook) - fp8 DoubleRow.

Problem: x (1, 131072, 256) f32, cluster_centers (1024, 256) f32.
For each cluster k: argmin_n ||x[n] - c[k]||^2, return that x row.

Device strategy (8 cores, x sharded along n after host sort by |x|^2):
  argmin d2 == argmax s - x2,  s = 2*x.c.
  fp8e4 DoubleRow matmuls (contraction 256 in ONE instruction) compute
  s-tilde for all (point, cluster) pairs into PSUM regions of
  [128 k x 1024 pts].  Drain is split across three resources:
    V-regions: DVE tensor_reduce [128,4,1024] -> 4 group maxima (1024-pt
               groups) written to bmax.
    A-regions: ACT evacuates PSUM -> fp16 SBUF, DMA ships the raw fp16
               scores to DRAM; the host folds them (full resolution).
  Host recovery: per-cluster flags from group maxima bounds (sorted-x2
  interval + fp8 noise window THETA); flagged groups rescored exactly
  (GEMM for V-groups, dumped-score shortlist for A-groups), fp64 refine,
  first-original-index tiebreak.
"""

import os
import sys

for _p in ("/opt/trn_rl_repo",):
    if os.path.isdir(_p) and _p not in sys.path:
        sys.path.append(_p)

import numpy as np
import ml_dtypes

import concourse.bass as bass
import concourse.bacc as bacc
import concourse.mybir as mybir
import concourse.tile as tile

NCORES = 8
N = 131072
F = 256
K = 1024
SH = N // NCORES            # 16384 points per core
RW = 2048                   # region width (points)
NGP = SH // RW              # 8 point-chunks per core
KT = K // 128               # 8 cluster tiles
NREG = NGP * KT             # 32 regions per core
GRP = 1024                  # host-bound group size
NGRPC = SH // GRP           # 16 groups per core
NGRP = NCORES * NGRPC       # 128 groups total

# Uniform 1024-pt units: unit u = ch*KT + kt (ch = 1024-pt chunk).
# V-units: DVE tensor_reduce -> bmax col.  A-units: ACT evac fp16 + DMA
# dump to host.  Both kinds double-buffered: RV x2 + RA x2 = 8 banks.
NU = NGRPC * KT             # 128 units per core
NV_TARGET = 61
_acc = 0
VA = []
for _u in range(NU):
    _acc += NV_TARGET
    if _acc >= NU:
        _acc -= NU
        VA.append("V")
    else:
        VA.append("A")
NV = VA.count("V")
NA = VA.count("A")
VIDX = {}
ASLOT = {}
for _u in range(NU):
    if VA[_u] == "V":
        VIDX[_u] = len(VIDX)
    else:
        ASLOT[_u] = len(ASLOT)

THETA = 26.0                # covers 2*E(fp8 score noise) + margin
DCAND = 26.0                # d-window for dump shortlists (~2E + margin)
TOPM = 16                   # fp32->fp64 refine width per (cluster, group)

FP8 = ml_dtypes.float8_e4m3
F16 = np.float16


def build_nc():
    nc = bacc.Bacc("TRN2", target_bir_lowering=False, debug=False,
                   num_devices=NCORES)

    xt8 = nc.dram_tensor("xt8", [128, 2, SH], mybir.dt.float8e4,
                         kind="ExternalInput")
    ct8 = nc.dram_tensor("ct8", [128, 2, K], mybir.dt.float8e4,
                         kind="ExternalInput")
    bmax_d = nc.dram_tensor("bmax", [128, NV], mybir.dt.float32,
                            kind="ExternalOutput")
    sdump_d = nc.dram_tensor("sdump", [NA, 128, 1024], mybir.dt.float16,
                             kind="ExternalOutput")

    with tile.TileContext(nc) as tc:
        with (
            tc.tile_pool(name="consts", bufs=1) as cpool,
            tc.tile_pool(name="psum", bufs=1, space="PSUM") as ppool,
            tc.tile_pool(name="dump", bufs=4) as dpool,
        ):
            # weights FIRST (every matmul depends on them), then one
            # tile per 1024-pt chunk so each unit's matmuls only wait
            # on their own chunk's DMA
            ct = cpool.tile([128, 2, K], mybir.dt.float8e4, tag="ct")
            nc.sync.dma_start(ct[:], ct8[:])
            xchunks = []
            for h in range(NGRPC):
                xc = cpool.tile([128, 2, GRP], mybir.dt.float8e4,
                                tag=f"xc{h}", name=f"xc{h}")
                nc.sync.dma_start(xc[:],
                                  xt8[:, :, h * GRP:(h + 1) * GRP])
                xchunks.append(xc)

            bmax_t = cpool.tile([128, NV], mybir.dt.float32, tag="bm")

            # PE warmup (p-state ramp, overlaps input DMA window)
            warm_w = cpool.tile([128, 2, 128], mybir.dt.float8e4, tag="ww")
            nc.gpsimd.memset(warm_w[:], 0.0)
            warm_x = cpool.tile([128, 2, 512], mybir.dt.float8e4, tag="wx")
            nc.gpsimd.memset(warm_x[:], 0.0)
            warm_ps = ppool.tile([128, 1024], mybir.dt.float32, tag="RV",
                                 bufs=2, name="warmps")
            for _ in range(6):
                nc.tensor.matmul(warm_ps[:, 0:512], lhsT=warm_w[:],
                                 rhs=warm_x[:], start=True, stop=True,
                                 perf_mode=mybir.MatmulPerfMode.DoubleRow)

            for ch in range(NGRPC):
                for kt in range(KT):
                    u = ch * KT + kt
                    base = ch * GRP
                    if VA[u] == "V":
                        R = ppool.tile([128, 1024], mybir.dt.float32,
                                       tag="RV", bufs=2, name=f"RV{u}")
                    else:
                        R = ppool.tile([128, 1024], mybir.dt.float32,
                                       tag="RA", bufs=2, name=f"RA{u}")
                    for b in range(2):
                        nc.tensor.matmul(
                            R[:, b * 512:(b + 1) * 512],
                            lhsT=ct[:, :, kt * 128:(kt + 1) * 128],
                            rhs=xchunks[ch][:, :, b * 512:(b + 1) * 512],
                            start=True, stop=True,
                            perf_mode=mybir.MatmulPerfMode.DoubleRow)
                    if VA[u] == "V":
                        nc.vector.tensor_reduce(
                            out=bmax_t[:, VIDX[u]:VIDX[u] + 1],
                            in_=R[:],
                            axis=mybir.AxisListType.X,
                            op=mybir.AluOpType.max)
                    else:
                        S = dpool.tile([128, 1024], mybir.dt.float16,
                                       tag="S")
                        nc.scalar.copy(S[:], R[:])
                        nc.sync.dma_start(sdump_d[ASLOT[u]], S[:])

            nc.sync.dma_start(bmax_d[:, :], bmax_t[:])

    nc.compile()
    return nc


def host_prep(x, cluster_centers):
    """Sort points by |x|^2; build fp8 device inputs."""
    x0 = np.ascontiguousarray(x[0], dtype=np.float32)        # (N, F)
    C = np.ascontiguousarray(cluster_centers, dtype=np.float32)
    x2 = np.einsum('nf,nf->n', x0.astype(np.float64),
                   x0.astype(np.float64))
    order = np.argsort(x2, kind="stable").astype(np.int64)
    xs_all = x0[order]
    x2s = x2[order]

    # ct8[p, i, k] = fp8(2*C[k, 128i+p])
    c2 = (2.0 * C).T.astype(FP8)                    # (F, K)
    ct8_np = np.ascontiguousarray(
        c2.reshape(2, 128, K).transpose(1, 0, 2))   # (128, 2, K)

    in_maps = []
    xs8_all = []
    for c in range(NCORES):
        xs = xs_all[c * SH:(c + 1) * SH]
        xs8 = xs.T.astype(FP8)                      # (F, SH)
        xs8_all.append(xs8)
        xt8_np = np.ascontiguousarray(
            xs8.reshape(2, 128, SH).transpose(1, 0, 2))  # (128, 2, SH)
        in_maps.append({"xt8": xt8_np, "ct8": ct8_np})
    return in_maps, x0, C, order, xs_all, x2s


def host_combine(results, x0, C, order, xs_all, x2s):
    """Exact argmin recovery from device group maxima + dumped scores."""
    x64s = xs_all.astype(np.float64)
    C64 = C.astype(np.float64)
    x2s_32 = x2s.astype(np.float32)

    # bm[k, g]: max of s-tilde over group g (1024 sorted pts), cluster k.
    # group g = c*NGRPC + ch; unit u = ch*KT + kt.
    bm = np.empty((K, NGRP), dtype=np.float32)
    # dumps[c][(ch, kt)] = (128, 1024) fp16 raw scores
    dumps = []
    for c in range(NCORES):
        bmax = np.asarray(results[c]["bmax"])        # (128, NV)
        sd = np.asarray(results[c]["sdump"])         # (NA, 128, 1024)
        sdf = sd.astype(np.float32).max(axis=2)      # (NA, 128)
        dmap = {}
        for ch in range(NGRPC):
            for kt in range(KT):
                u = ch * KT + kt
                rows = slice(kt * 128, (kt + 1) * 128)
                g = c * NGRPC + ch
                if VA[u] == "V":
                    bm[rows, g] = bmax[:, VIDX[u]]
                else:
                    dmap[(ch, kt)] = sd[ASLOT[u]]
                    bm[rows, g] = sdf[ASLOT[u]]
        dumps.append(dmap)

    gb = np.arange(NGRP) * GRP
    x2min = x2s[gb].astype(np.float32)
    x2max = x2s[gb + GRP - 1].astype(np.float32)

    ub = bm - x2min[None, :]
    lb = bm - x2max[None, :]
    win_lb = lb.max(axis=1)
    flags = ub >= (win_lb[:, None] - THETA)          # (K, NGRP)

    ks_idx, gs_idx = np.nonzero(flags)
    pair_clusters = [[] for _ in range(NGRP)]
    for kk, g in zip(ks_idx, gs_idx):
        pair_clusters[g].append(kk)

    # candidate accumulation: (cluster, point candidates)
    cand_k = []
    cand_srt = []

    for g, ks in enumerate(pair_clusters):
        if not ks:
            continue
        ksa = np.asarray(ks)
        base = g * GRP
        c, ch = divmod(g, NGRPC)
        kt_of = ksa // 128
        gemm_ks = []
        for kt in np.unique(kt_of):
            u = ch * KT + kt
            kss = ksa[kt_of == kt]
            if VA[u] == "A":
                s = dumps[c][(ch, kt)]               # (128, 1024) fp16
                p = kss - kt * 128
                dt_ = x2s_32[base:base + GRP][None, :] - \
                    s[p].astype(np.float32)          # (|kss|, 1024)
                part = np.argpartition(dt_, TOPM, axis=1)[:, :TOPM]
                for j, kk in enumerate(kss):
                    cand_k.append(np.full(TOPM, kk))
                    cand_srt.append(base + part[j])
            else:
                gemm_ks.append(kss)
        if gemm_ks:
            ksg = np.concatenate(gemm_ks)
            pts = xs_all[base:base + GRP]
            d32 = x2s_32[base:base + GRP, None] - \
                2.0 * (pts @ C[ksg].T)               # (1024, |ksg|)
            part = np.argpartition(d32, TOPM, axis=0)[:TOPM]
            for j, kk in enumerate(ksg):
                cand_k.append(np.full(TOPM, kk))
                cand_srt.append(base + part[:, j])

    ck = np.concatenate(cand_k)                      # (M,)
    cs = np.concatenate(cand_srt)                    # (M,)

    # fp64 exact refine on all candidates
    dv = x2s[cs] - 2.0 * np.einsum('mf,mf->m', x64s[cs], C64[ck])
    ids = order[cs]
    # pick per cluster: min dv, tiebreak min original index
    o = np.lexsort((ids, dv, ck))
    cko = ck[o]
    first = np.ones(len(cko), dtype=bool)
    first[1:] = cko[1:] != cko[:-1]
    sel = o[first]
    kk_sel = ck[sel]
    best_idx = np.zeros(K, dtype=np.int64)
    best_idx[kk_sel] = ids[sel]
    assert len(kk_sel) == K, f"only {len(kk_sel)} clusters covered"

    return x0[best_idx][None].astype(np.float32)


_NC_CACHE = {}


def kernel(x, cluster_centers):
    from concourse.bass_utils import run_bass_kernel_spmd

    if "nc" not in _NC_CACHE:
        _NC_CACHE["nc"] = build_nc()
    nc = _NC_CACHE["nc"]

    in_maps, x0, C, order, xs_all, x2s = host_prep(x, cluster_centers)
    res = run_bass_kernel_spmd(nc, in_maps, list(range(NCORES)))
    return host_combine([res.results[c] for c in range(NCORES)],
                        x0, C, order, xs_all, x2s)
